# revision 2
# baseline (speedup 1.0000x reference)
"""Multi-head GAT layer on 8 Trainium2 NeuronCores (Bass/Tile SPMD kernel), v2.

Strategy (edge-parallel, target-sharded, gather-throughput optimized):
  - Edges sorted by target, sharded across 8 cores by contiguous target
    ranges (N/8 nodes each): softmax + aggregation are core-local.
  - Node stream is ROTATED per core on the host: core c's nft input
    holds node (c*NPC + j) % N at column j, so every core's own targets
    are local rows [0, NPC) -- static addresses in the shared SPMD
    program.  Per-core gather indices absorb the rotation.
  - Phase 1 (replicated compute on every core): one bf16 PE pass over
    the rotated node features builds a per-node table row
      [ h (128) | s2 (8) | s1 (8) | junk ]  (bf16, 512B rows)
    stored WRAPPED (local j -> flat row (j%128)*NK + j//128) so table
    writes are contiguous per partition (line-rate HWDGE).  Rows for
    the core's own targets (j < NPC) are also written to a compact
    node-major own-table, giving the tail contiguous self reads (no
    indirect DMA).
  - Phase 2: per 128-target block, edge slots (padded to 128-slot
    tiles, sorted by remapped src id) are fetched with dma_gather on 4
    round-robin SWDGE queues with deep G buffering so all 4 queues
    drain concurrently (the gather is descriptor-latency-bound: ~3.3
    ns/descriptor at 4 queues vs ~10.7 single-queue).  One-hot matrices
    (slot->target and its transpose) are precomputed on the host in fp8
    and streamed via HWDGE -- no on-chip one-hot build, no PE
    transposes.  Per tile, PE matmuls compute the per-slot target score
    broadcast (zps = ohT @ s1) and accumulate the weighted message sum
    plus the softmax denominator in one PSUM pass (rhs = [msg|ex]).
    Softmax division, skip term (deg * h_own), and ELU finalize per
    block, emitted one block late to overlap PSUM waits.
"""

import numpy as np

N_CORES = 8
_last_results = None  # BassKernelResults of the most recent run (for harnesses)


def _install_ntff_hook():
    """Register the axon NTFF profiling hook if the image lacks antenv.axon_hooks."""
    import sys, types
    try:
        from antenv.axon_hooks import get_axon_ntff_profile_hook  # noqa: F401
        return
    except ImportError:
        pass
    try:
        mod = types.ModuleType("antenv.axon_hooks")
        holder = [None]
        mod.set_axon_ntff_profile_hook = lambda h: holder.__setitem__(0, h)
        mod.get_axon_ntff_profile_hook = lambda: holder[0]
        sys.modules["antenv.axon_hooks"] = mod
        from trn_agent_boot.trn_boot import _ntff_profile_via_ctypes
        mod.set_axon_ntff_profile_hook(
            _ntff_profile_via_ctypes("/opt/axon/libaxon_pjrt.so"))
    except Exception:
        sys.modules.pop("antenv.axon_hooks", None)


def kernel(node_features, edge_index, W, b, a):
    return gat_multicore(
        np.asarray(node_features, dtype=np.float32),
        np.asarray(edge_index, dtype=np.int32),
        np.asarray(W, dtype=np.float32),
        np.asarray(b, dtype=np.float32),
        np.asarray(a, dtype=np.float32),
    )


def gat_multicore(nf, ei, W, b, a, slope=0.2):
    import sys
    if "/opt/trn_rl_repo" not in sys.path:
        sys.path.insert(0, "/opt/trn_rl_repo")
    import ml_dtypes
    import concourse.bacc as bacc
    import concourse.tile as tile
    import concourse.mybir as mybir
    from concourse import library_config
    from concourse.bass_utils import run_bass_kernel_spmd
    from contextlib import ExitStack

    fp32 = mybir.dt.float32
    bf16 = mybir.dt.bfloat16
    fp8 = mybir.dt.float8e4
    i16 = mybir.dt.int16
    AF = mybir.ActivationFunctionType
    OP = mybir.AluOpType
    bfnp = ml_dtypes.bfloat16
    f8np = ml_dtypes.float8_e4m3

    N, F_IN = nf.shape
    E = ei.shape[1]
    HF = W.shape[0]               # H * F_OUT
    F_OUT = a.shape[0] // 2
    H = HF // F_OUT
    assert F_IN == 128 and HF == 128, "kernel assumes 128 in/out features"
    assert N % N_CORES == 0
    NPC = N // N_CORES            # targets per core
    NBLK = (NPC + 127) // 128     # 128-target blocks per core
    NK = (N + 127) // 128         # node tiles (wrapped table columns)
    NWR = NK * 128                # padded node count
    GRP = 8                       # max tiles per gather group
    ROW = 256                     # bf16 elements per table row (512 B)
    SW = HF + 2 * H               # 144: phase-1 psum width [h|s2|s1]
    SPAN = 32000                  # max int16 index span per gather group

    # ---------------- host prep: weights ----------------
    WT = np.ascontiguousarray(W.T)                       # [F_IN, HF]
    # extra matmul columns: [s2 (a2) | s1 (a1)] per head
    A12 = np.zeros((HF, 2 * H), dtype=np.float32)
    for hd in range(H):
        A12[hd * F_OUT:(hd + 1) * F_OUT, hd] = a[F_OUT:]        # s2
        A12[hd * F_OUT:(hd + 1) * F_OUT, H + hd] = a[:F_OUT]    # s1
    rhs_ext = np.concatenate([WT, WT @ A12], axis=1).astype(np.float32)
    b12 = (b @ A12).astype(np.float32)
    b_ext = np.concatenate([b, b12]).astype(np.float32)  # [144]
    b_rep = np.broadcast_to(b_ext, (128, SW)).copy()
    b_is_zero = not np.any(b_ext)
    nfT = np.ascontiguousarray(nf.T).astype(bfnp)        # [128, N]

    # local id j -> wrapped flat table row
    def remap(j):
        return (j % 128) * NK + j // 128

    # ---------------- host prep: graph structure ----------------
    src, tgt = ei[0].astype(np.int64), ei[1].astype(np.int64)
    order = np.argsort(tgt, kind="stable")
    ssrc, stgt = src[order], tgt[order]
    deg_full = np.bincount(tgt, minlength=N).astype(np.float32)

    blk_bounds = []
    for c in range(N_CORES):
        bounds = [c * NPC + bb * 128 for bb in range(NBLK)] + [(c + 1) * NPC]
        blk_bounds.append(np.searchsorted(stgt, bounds))
    cnt = np.array([[blk_bounds[c][bb + 1] - blk_bounds[c][bb]
                     for bb in range(NBLK)] for c in range(N_CORES)])
    n_edge_tiles = np.maximum(1, (cnt.max(axis=0) + 127) // 128)
    NT = int(n_edge_tiles.sum())
    t_ofs_blk = np.concatenate([[0], np.cumsum(n_edge_tiles)]).astype(int)
    MAXT = int(n_edge_tiles.max())

    # Per-core slot arrays; tile t slot p = slot index t*128+p of the block.
    srcs_all = np.zeros((N_CORES, 128, NT), dtype=np.int64)   # remapped ids
    rowid_np = np.full((N_CORES, 128, NT), -1, dtype=np.int64)
    deg_own = np.zeros((N_CORES, 128, NBLK), dtype=np.float32)
    for c in range(N_CORES):
        for bb in range(NBLK):
            lo, hi = blk_bounds[c][bb], blk_bounds[c][bb + 1]
            nslot = hi - lo
            base_node = c * NPC + bb * 128
            nrows = min(128, (c + 1) * NPC - base_node)
            t0 = int(t_ofs_blk[bb])
            net = int(n_edge_tiles[bb])
            ne = net * 128
            if nslot > 0:
                rsrc = remap((ssrc[lo:hi] - c * NPC) % N)
                o2 = np.argsort(rsrc, kind="stable")
                s_blk = rsrc[o2]
                fl_s = np.full(ne, int(s_blk[-1]), dtype=np.int64)
                fl_r = np.full(ne, -1, dtype=np.int64)
                fl_s[:nslot] = s_blk
                fl_r[:nslot] = stgt[lo:hi][o2] - base_node
                srcs_all[c, :, t0:t0 + net] = fl_s.reshape(net, 128).T
                rowid_np[c, :, t0:t0 + net] = fl_r.reshape(net, 128).T
            own = np.arange(128)
            deg_own[c, :, bb] = np.where(
                own < nrows, deg_full[np.minimum(base_node + own, N - 1)], 0.0)
    # blocks empty on some core but not others: borrow a non-empty core's
    # pad value so the cross-core span stays bounded
    for bb in range(NBLK):
        t0 = int(t_ofs_blk[bb])
        net = int(n_edge_tiles[bb])
        nonempty = [c for c in range(N_CORES) if cnt[c][bb] > 0]
        if nonempty and len(nonempty) < N_CORES:
            ref = int(srcs_all[nonempty[0], 0, t0])
            for c in range(N_CORES):
                if cnt[c][bb] == 0:
                    srcs_all[c, :, t0:t0 + net] = ref

    # Gather groups: consecutive tiles of one block, <= GRP tiles,
    # cross-core remapped span <= SPAN.  Base per group = cross-core min.
    groups = []          # (block, tile_lo, n_tiles, base)
    for bb in range(NBLK):
        net = int(n_edge_tiles[bb])
        t0 = int(t_ofs_blk[bb])
        t = 0
        while t < net:
            best = 1
            for w in range(2, min(GRP, net - t) + 1):
                sl = srcs_all[:, :, t0 + t:t0 + t + w]
                if sl.max() - sl.min() > SPAN:
                    break
                best = w
            sl = srcs_all[:, :, t0 + t:t0 + t + best]
            assert sl.max() - sl.min() <= SPAN, "single tile span too large"
            groups.append((bb, t, best, int(sl.min())))
            t += best

    g_cols = [(g[2] * 128) // 16 for g in groups]
    g_col_ofs = np.concatenate([[0], np.cumsum(g_cols)]).astype(int)
    IDXC = int(g_col_ofs[-1])
    idx16_np = np.zeros((N_CORES, 128, IDXC), dtype=np.int16)
    for c in range(N_CORES):
        for gi, (bb, tl, w, base) in enumerate(groups):
            t0 = int(t_ofs_blk[bb]) + tl
            rel = (srcs_all[c, :, t0:t0 + w] - base).astype(np.int16)  # [128, w]
            flat = rel.T.reshape(-1)                 # slot order t*128+p
            wrapped = flat.reshape(-1, 16).T         # [16, w*128/16]
            idx16_np[c, :, g_col_ofs[gi]:g_col_ofs[gi + 1]] = np.tile(wrapped, (8, 1))

    # One-hot matrices (static, fp8), one fused tensor per core with per-block
    # layout [oh tiles | ohT tiles]: oh[p=slot, tgt], ohT[p=tgt, slot].
    # Column offset of block bb: oh at 2*t0*128, ohT at (2*t0 + net)*128.
    one = np.float32(1.0).astype(f8np)
    ohc_np = np.zeros((N_CORES, 128, 2 * NT * 128), dtype=f8np)
    for c in range(N_CORES):
        for bb in range(NBLK):
            t0 = int(t_ofs_blk[bb])
            net = int(n_edge_tiles[bb])
            sl = rowid_np[c][:, t0:t0 + net]
            p_arr, t_arr = np.nonzero(sl >= 0)
            g_arr = sl[p_arr, t_arr]
            o_ofs = 2 * t0 * 128
            ohc_np[c][p_arr, o_ofs + t_arr * 128 + g_arr] = one
            ohc_np[c][g_arr, o_ofs + (net + t_arr) * 128 + p_arr] = one

    # Rotated node features per core: column j holds node (c*NPC + j) % N.
    nft_rot = []
    for c in range(N_CORES):
        arr = np.zeros((128, NWR), dtype=bfnp)
        r = c * NPC
        arr[:, :N - r] = nfT[:, r:]
        arr[:, N - r:N] = nfT[:, :r]
        nft_rot.append(arr)

    # ---------------- build the SPMD program ----------------
    nc = bacc.Bacc("TRN2", target_bir_lowering=False, debug=False,
                   num_devices=N_CORES, num_swdge_queues=4)

    nft_d = nc.dram_tensor("nft", [128, NWR], bf16, kind="ExternalInput").ap()
    wte_d = nc.dram_tensor("wte", [128, SW], bf16, kind="ExternalInput").ap()
    brep_d = nc.dram_tensor("brep", [128, SW], fp32, kind="ExternalInput").ap()
    idx16_d = nc.dram_tensor("idx16", [128, IDXC], i16, kind="ExternalInput").ap()
    ohc_d = nc.dram_tensor("ohc", [128, 2 * NT * 128], fp8, kind="ExternalInput").ap()
    deg_d = nc.dram_tensor("deg", [128, NBLK], fp32, kind="ExternalInput").ap()

    tab_d = nc.dram_tensor("tab", [NWR, ROW], bf16).ap()
    out_d = nc.dram_tensor("out", [NPC, HF], fp32, kind="ExternalOutput").ap()
    import os as _os
    _dbg = bool(_os.environ.get("GAT_DEBUG"))
    dbg_d = (nc.dram_tensor("dbg", [NBLK * 128, ROW], bf16,
                            kind="ExternalOutput").ap() if _dbg else None)
    dbg2_d = (nc.dram_tensor("dbg2", [128, ROW], bf16,
                             kind="ExternalOutput").ap() if _dbg else None)

    tabw = tab_d.rearrange("(p k) r -> p k r", p=128)   # wrapped write view

    with tile.TileContext(nc) as tc:
        with ExitStack() as ctx:
            cpool = ctx.enter_context(tc.tile_pool(name="consts", bufs=1))
            p1 = ctx.enter_context(tc.tile_pool(name="p1", bufs=3))
            p1ps = ctx.enter_context(tc.tile_pool(name="p1ps", bufs=1, space="PSUM"))
            gp = ctx.enter_context(tc.tile_pool(name="gather", bufs=6))
            ohp = ctx.enter_context(tc.tile_pool(name="ohp", bufs=3))
            sp_ = ctx.enter_context(tc.tile_pool(name="selfp", bufs=3))
            mp = ctx.enter_context(tc.tile_pool(name="meta", bufs=3))
            ps_acc = ctx.enter_context(tc.tile_pool(name="ps_acc", bufs=2, space="PSUM"))
            ps_accd = ctx.enter_context(tc.tile_pool(name="ps_accd", bufs=2, space="PSUM"))
            ps_z = ctx.enter_context(tc.tile_pool(name="ps_z", bufs=2, space="PSUM"))
            fin = ctx.enter_context(tc.tile_pool(name="fin", bufs=2))

            nc.gpsimd.load_library(library_config.mlp)

            def touch(tile_ap):
                # 1-elem in-place DVE copy: serializes the tile's DMA load
                # behind a DVE op so later readers inherit a single DVE
                # dep (instructions carry only one wait slot; extra DMA
                # deps are dropped by the tracker).
                nc.vector.tensor_copy(tile_ap[0:1, 0:1], tile_ap[0:1, 0:1])

            wte_sb = cpool.tile([128, SW], bf16)
            nc.sync.dma_start(wte_sb[:], wte_d[:])
            idx_sb = cpool.tile([128, IDXC], i16)
            nc.sync.dma_start(idx_sb[:], idx16_d[:])
            deg_sb = cpool.tile([128, NBLK], fp32)
            nc.sync.dma_start(deg_sb[:], deg_d[:])
            touch(wte_sb)
            touch(idx_sb)
            touch(deg_sb)
            if not b_is_zero:
                brep_sb = cpool.tile([128, SW], fp32)
                nc.sync.dma_start(brep_sb[:], brep_d[:])
                touch(brep_sb)

            # ---------- phase 1: wrapped h table (rotated node stream) ----------
            CH = 2048          # nodes per nfc load
            GK = 4             # node tiles per psum group (1KB pitch: no
                               # matmul output ever crosses a PSUM bank)
            HG = 8             # node tiles per hrow / table write
            pend = None        # delayed table write: (hrow, kb0, nk)
            for j0 in range(0, NWR, CH):
                w = min(CH, NWR - j0)
                nfc = p1.tile([128, CH], bf16, tag="nfc")
                nc.sync.dma_start(nfc[:, :w], nft_d[:, j0:j0 + w])
                touch(nfc)
                for h0 in range(0, w, HG * 128):
                    hw2 = min(HG * 128, w - h0)
                    nhk = (hw2 + 127) // 128
                    hrow = p1.tile([128, HG, ROW], bf16, tag="hrow")
                    for k0 in range(h0, h0 + hw2, GK * 128):
                        kw2 = min(GK * 128, w - k0)
                        nk = (kw2 + 127) // 128
                        ps = p1ps.tile([128, GK, 256], fp32, space="PSUM",
                                       tag="p1ps")
                        for k in range(nk):
                            kk = k0 + k * 128
                            kw = min(128, w - kk)
                            nc.tensor.matmul(ps[:kw, k, 0:SW],
                                             lhsT=nfc[:, kk:kk + kw],
                                             rhs=wte_sb[:], start=True, stop=True)
                        kof = (k0 - h0) // 128
                        if b_is_zero:
                            nc.vector.tensor_copy(
                                hrow[:, kof:kof + nk, 0:SW], ps[:, :nk, 0:SW])
                        else:
                            nc.vector.tensor_tensor(
                                out=hrow[:, kof:kof + nk, 0:SW],
                                in0=ps[:, :nk, 0:SW],
                                in1=brep_sb[:].unsqueeze(1).broadcast_to(
                                    [128, nk, SW]),
                                op=OP.add)
                    if pend is not None:
                        ph, pkb, pnk = pend
                        nc.sync.dma_start(tabw[:, pkb:pkb + pnk, :], ph[:, :pnk, :])
                    pend = (hrow, h0 // 128 + j0 // 128, nhk)
            ph, pkb, pnk = pend
            nc.sync.dma_start(tabw[:, pkb:pkb + pnk, :], ph[:, :pnk, :])

            # ---------- phase 2: edge processing ----------
            blk_state = {}

            def emit_gathers(bb, qn0):
                t0 = int(t_ofs_blk[bb])
                qn = qn0
                G = gp.tile([128, MAXT, ROW], bf16, tag="G")
                for gi, (gbb, tl, wdt, base) in enumerate(groups):
                    if gbb != bb:
                        continue
                    nc.gpsimd.dma_gather(
                        out_ap=G[:, tl:tl + wdt, :],
                        in_ap=tab_d[base:, :],
                        idxs_ap=idx_sb[:, g_col_ofs[gi]:g_col_ofs[gi + 1]],
                        num_idxs=wdt * 128, num_idxs_reg=wdt * 128,
                        elem_size=ROW, queue_num=qn % 4)
                    qn += 1
                return G, qn

            def emit_compute(bb, G):
                net = int(n_edge_tiles[bb])
                t0 = int(t_ofs_blk[bb])

                self_sb = sp_.tile([128, ROW], bf16, tag="self")
                nc.sync.dma_start(self_sb[:], tabw[:, bb, :])
                touch(self_sb)

                # fused [oh | ohT] one-hot load for the block
                ohc_sb = ohp.tile([128, 2 * MAXT, 128], fp8, tag="ohc")
                o_ofs = 2 * t0 * 128
                nc.sync.dma_start(ohc_sb[:, :2 * net, :],
                                  ohc_d[:, o_ofs:o_ofs + 2 * net * 128]
                                  .rearrange("p (t s) -> p t s", t=2 * net))
                touch(ohc_sb[:, 0, :])
                oh_sb = ohc_sb[:, 0:net, :]
                ohT_sb = ohc_sb[:, net:2 * net, :]

                s1_blk = self_sb[:, SW - H:SW]           # [128, H] bf16

                zps = ps_z.tile([128, MAXT, H], fp32, space="PSUM", tag="zps")
                for t in range(net):
                    nc.tensor.matmul(zps[:, t, :], lhsT=ohT_sb[:, t, :],
                                     rhs=s1_blk, start=True, stop=True)

                # z = zps + s2_src ; ex = exp(leakyrelu(z))
                z_sb = mp.tile([128, MAXT, H], fp32, tag="z_sb")
                nc.vector.tensor_tensor(out=z_sb[:, :net, :], in0=zps[:, :net, :],
                                        in1=G[:, :net, HF:HF + H], op=OP.add)
                ex = mp.tile([128, MAXT, H], bf16, tag="ex")
                nc.vector.scalar_tensor_tensor(
                    out=ex[:, :net, :], in0=z_sb[:, :net, :],
                    scalar=slope, in1=z_sb[:, :net, :], op0=OP.mult, op1=OP.max)
                nc.scalar.activation(ex[:, :net, :], ex[:, :net, :], AF.Exp)
                # msg = ex * h_src
                msg = mp.tile([128, MAXT, HF], bf16, tag="msg")
                nc.vector.tensor_tensor(
                    out=msg[:, :net, :], in0=G[:, :net, 0:HF],
                    in1=ex[:, :net, :].unsqueeze(3).broadcast_to(
                        [128, net, H, F_OUT]),
                    op=OP.mult)

                acc = ps_acc.tile([128, HF], fp32, space="PSUM", tag="acc")
                accd = ps_accd.tile([128, H], fp32, space="PSUM", tag="accd")
                for t in range(net):
                    nc.tensor.matmul(acc[:, :], lhsT=oh_sb[:, t, :],
                                     rhs=msg[:, t, :],
                                     start=(t == 0), stop=(t == net - 1))
                    nc.tensor.matmul(accd[:, :], lhsT=oh_sb[:, t, :],
                                     rhs=ex[:, t, :],
                                     start=(t == 0), stop=(t == net - 1))

                blk_state[bb] = (acc, accd, self_sb)

            def emit_tail(bb):
                base_row = bb * 128
                nrows = min(128, NPC - base_row)
                acc, accd, self_sb = blk_state.pop(bb)

                rec = fin.tile([128, H], fp32, tag="rec")
                nc.vector.tensor_scalar_add(out=rec[:, :], in0=accd[:, :],
                                            scalar1=1e-30)
                nc.vector.reciprocal(rec[:, :], rec[:, :])
                nrm = fin.tile([128, HF], fp32, tag="nrm")
                nc.vector.tensor_tensor(
                    out=nrm[:, :], in0=acc[:, 0:HF],
                    in1=rec[:].unsqueeze(2).broadcast_to([128, H, F_OUT]),
                    op=OP.mult)
                # += deg * h_own (exact skip term)
                nc.vector.scalar_tensor_tensor(
                    out=nrm[:, :], in0=self_sb[:, 0:HF],
                    scalar=deg_sb[:, bb:bb + 1],
                    in1=nrm[:, :], op0=OP.mult, op1=OP.add)
                # ELU = max(x,0) + exp(min(x,0)) - 1
                neg = fin.tile([128, HF], fp32, tag="neg")
                nc.vector.tensor_scalar_min(out=neg[:, :], in0=nrm[:, :], scalar1=0.0)
                nc.scalar.activation(neg[:, :], neg[:, :], AF.Exp)
                pos = fin.tile([128, HF], fp32, tag="pos")
                nc.vector.tensor_scalar_max(out=pos[:, :], in0=nrm[:, :], scalar1=0.0)
                res = fin.tile([128, HF], fp32, tag="res")
                nc.vector.scalar_tensor_tensor(
                    out=res[:, :], in0=neg[:, :], scalar=-1.0, in1=pos[:, :],
                    op0=OP.add, op1=OP.add)
                nc.scalar.dma_start(out_d[base_row:base_row + nrows, :],
                                    res[:nrows, :])

            if _dbg:
                dbg_sb = cpool.tile([128, ROW], bf16)
                for bb in range(NBLK):
                    nc.sync.dma_start(dbg_sb[:], tabw[:, bb, :])
                    nc.sync.dma_start(
                        dbg_d[bb * 128:(bb + 1) * 128, :], dbg_sb[:])
            qn = 0
            for bb in range(NBLK):
                G, qn = emit_gathers(bb, qn)
                emit_compute(bb, G)
                if bb > 0:
                    emit_tail(bb - 1)
            emit_tail(NBLK - 1)
            if _dbg:
                dbg2_sb = cpool.tile([128, ROW], bf16)
                nc.sync.dma_start(dbg2_sb[:], tabw[:, 3, :])
                touch(dbg2_sb)
                nc.sync.dma_start(dbg2_d[:, :], dbg2_sb[:])

    nc.compile()

    in_maps = []
    for c in range(N_CORES):
        in_maps.append({
            "nft": nft_rot[c], "wte": rhs_ext.astype(bfnp), "brep": b_rep,
            "idx16": idx16_np[c], "ohc": ohc_np[c],
            "deg": deg_own[c],
        })
    import os
    trace = bool(os.environ.get("GAT_TRACE"))
    if trace:
        _install_ntff_hook()
    res = run_bass_kernel_spmd(nc, in_maps, list(range(N_CORES)), trace=trace)
    global _last_results
    _last_results = res
    out = np.concatenate([res.results[c]["out"] for c in range(N_CORES)], axis=0)
    if _dbg:
        global _dbg_own, _dbg_col3
        _dbg_own = [res.results[c]["dbg"] for c in range(N_CORES)]
        _dbg_col3 = [res.results[c]["dbg2"] for c in range(N_CORES)]
    return out
